# revision 37
# baseline (speedup 1.0000x reference)
"""Multi-head attention Bass kernel for Trainium2, SPMD over 8 NeuronCores.

Problem: B=4, S=2048, D=1024, 16 heads x 64. Sharding: core = (batch b, head-group hg)
with b in 0..3, hg in 0..1 -> each core computes 8 heads of one batch.

Design (cost-model driven, fp16 end-to-end):
  - ScalarE's exp is the hard floor: 256 activations of [128(k), 2(head),
    512(q)] PSUM fp32 -> p fp16, ~1.04us each = ~266us. Everything else is
    arranged to hide under it.
  - scores (PE): per (kc, head) one fp16 matmul K=64 -> s[k, q] in PSUM.
  - AV is Q-MAJOR: O[q, dh] accumulates with M=128 q-positions on PSUM
    partitions and only N=66 columns (64 dh + a ones column that picks up
    the softmax denominator for free + 1 pad for 8B alignment). lhsT is
    the p tile (stationary), rhs is V-augmented [k, 66]. Cost: 66 cycles
    per (kc, head, q-128-chunk) -> ~58us total, half of the k-major form,
    and the denominator lands per-partition-aligned with q so normalize is
    reciprocal + tensor_scalar_mul per chunk - no cross-partition traffic.
  - Four accumulation chains share each PSUM bank; only the chain writing
    first uses start=True (clears the whole bank's has_written bits), the
    others overwrite-where-unset.
  - All deferred work (V projection chunks, AV+finalize, later Q/K
    projections) sits in queues annotated with the earliest "slot" (ACT
    count) at which its input DMA will have landed, so a not-yet-ready
    instruction never enters the PE FIFO ahead of the scores matmuls that
    feed ScalarE. AV closures additionally gate on their V chunk being
    emitted; finalize closures ride the same queue so o_ps frees in order.
  - inputs stream as column-sliced DMAs in consumption order (the DMA
    engine pool is serial in the cost model): wq, wk, xk0, xq0, xk1, xv0,
    xk2, xv1, xk3, xq1, xv2, xv3, xq2, xq3.

PSUM (8 banks): scores 2bufs x [128,2,512] f32 = 4, O accum 2 (2qc x 2h x 66
x 2 banks), projection staging 2.
"""
import numpy as np
import ml_dtypes
from contextlib import ExitStack

import concourse.tile as tile
import concourse.mybir as mybir
from concourse import bacc
from concourse.bass_utils import run_bass_kernel_spmd

P = 128
DH = 64
F16 = mybir.dt.float16
F32 = mybir.dt.float32

AV_START = 2          # earliest kc for AV draining (first block)
AV_RATE = 6           # max AV/finalize closures drained per kc
V_RATE = 3            # max V-projection closures drained per kc
# earliest global slot for V chunk group c//4 (when xv quarter c//4 landed)
V_SLOTS = (13, 16, 19, 22)
PQ_RATE = 3           # max projection closures drained per kc


def build_attention(S=2048, D=1024, HPC=8, loop_n=1, pops=PQ_RATE):
    """Build the per-core SPMD program. HPC = heads per core (even).

    loop_n > 1 wraps the whole body in a hardware loop (for timing)."""
    DC = D // P        # D chunks of 128
    KC = S // P        # k chunks of 128
    NQ = S // 512      # q blocks of 512
    HP = HPC // 2      # head pairs
    CW = HPC * DH      # core output width
    ACT_SCALE = 1.0 / float(np.sqrt(DH))

    nc = bacc.Bacc("TRN2")
    xq = nc.dram_tensor("xq", [P, DC, S], F16, kind="ExternalInput")
    xk = nc.dram_tensor("xk", [P, DC, S], F16, kind="ExternalInput")
    xv = nc.dram_tensor("xv", [P, DC, S], F16, kind="ExternalInput")
    wq = nc.dram_tensor("wq", [P, DC, CW], F16, kind="ExternalInput")
    wk = nc.dram_tensor("wk", [P, DC, CW], F16, kind="ExternalInput")
    wv = nc.dram_tensor("wv", [P, DC, CW], F16, kind="ExternalInput")
    out = nc.dram_tensor("out", [HPC, P, NQ, 4, DH], F32, kind="ExternalOutput")

    with tile.TileContext(nc) as tc, ExitStack() as ctx:
        xpool = ctx.enter_context(tc.tile_pool(name="x", bufs=1))
        wpool = ctx.enter_context(tc.tile_pool(name="w", bufs=1))
        vpool = ctx.enter_context(tc.tile_pool(name="v", bufs=1))
        qkpool = ctx.enter_context(tc.tile_pool(name="qk", bufs=3))
        ppool = ctx.enter_context(tc.tile_pool(name="p", bufs=20))
        rpool = ctx.enter_context(tc.tile_pool(name="r", bufs=4))
        opool = ctx.enter_context(tc.tile_pool(name="ob", bufs=2))
        otpool = ctx.enter_context(tc.tile_pool(name="ot", bufs=2))
        ps_s = ctx.enter_context(tc.tile_pool(name="ps_s", bufs=2, space="PSUM"))
        ps_o = ctx.enter_context(tc.tile_pool(name="ps_o", bufs=1, space="PSUM"))
        ps_m = ctx.enter_context(tc.tile_pool(name="ps_m", bufs=2, space="PSUM"))

        xs, ws = {}, {}
        vta = None
        slot = [0]           # global ACT counter
        vta_done = [False] * KC

        def emit_loads():
            nonlocal vta
            for name in ("q", "k", "v"):
                ws[name] = wpool.tile([P, DC, CW], F16, tag="w" + name,
                                      name="w" + name)
                xs[name] = xpool.tile([P, DC, S], F16, tag="x" + name,
                                      name="x" + name)

            def ld(t, dram, c0, c1):
                nc.sync.dma_start(t[:, :, c0:c1], dram[:, :, c0:c1])

            # DMA order = consumption order (DMA engine pool is serial).
            # hp0's weight columns first; the rest of W after the k/v bulk.
            nc.sync.dma_start(ws["q"][:, :, 0:P], wq[:, :, 0:P])
            nc.sync.dma_start(ws["k"][:, :, 0:P], wk[:, :, 0:P])
            ld(xs["q"], xq, 0, 512)
            ld(xs["k"], xk, 0, 512)
            ld(xs["k"], xk, 512, 1024)
            ld(xs["k"], xk, 1024, 1536)
            nc.sync.dma_start(ws["v"][:], wv[:])
            ld(xs["k"], xk, 1536, 2048)
            ld(xs["v"], xv, 0, 512)
            ld(xs["q"], xq, 512, 1024)
            ld(xs["v"], xv, 512, 1024)
            ld(xs["v"], xv, 1024, 1536)
            ld(xs["v"], xv, 1536, 2048)
            nc.sync.dma_start(ws["q"][:, :, P:CW], wq[:, :, P:CW])
            nc.sync.dma_start(ws["k"][:, :, P:CW], wk[:, :, P:CW])
            ld(xs["q"], xq, 1024, 1536)
            ld(xs["q"], xq, 1536, 2048)
            # V-augmented rhs: [kpos, kc, ch, 66] = V | 1.0 | 0 pad
            vta = vpool.tile([P, KC, HPC, 66], F16, tag="V", name="vta")
            nc.vector.memset(vta[:, :, :, 64], 1.0)
            nc.vector.memset(vta[:, :, :, 65], 0.0)

        def v_closures(kc, min_slot):
            pstate = {}

            def mk(dc):
                def f():
                    if dc == 0:
                        pstate["pv"] = ps_m.tile([P, 512], F32,
                                                 tag="proj", name="pv")
                    nc.tensor.matmul(
                        pstate["pv"][:, :CW],
                        xs["v"][:, dc, kc * P : (kc + 1) * P],
                        ws["v"][:, dc, :],
                        start=(dc == 0),
                        stop=(dc == DC - 1),
                    )
                    if dc == DC - 1:
                        nc.vector.tensor_copy(
                            vta[:, kc, :, 0:DH],
                            pstate["pv"][:, :CW].rearrange(
                                "p (h d) -> p h d", d=DH),
                        )
                        vta_done[kc] = True
                return f

            return [(min_slot, mk(d)) for d in range(DC)]

        def new_qk(which):
            return qkpool.tile([P, S], F16, tag=which, name=which + "t")

        def proj_qk_chunk(t, which, hp, qb):
            pp = ps_m.tile([P, 512], F32, tag="proj", name="pp")
            for dc in range(DC):
                nc.tensor.matmul(
                    pp[:],
                    ws[which][:, dc, hp * P : (hp + 1) * P],
                    xs[which][:, dc, qb * 512 : (qb + 1) * 512],
                    start=(dc == 0),
                    stop=(dc == DC - 1),
                )
            nc.vector.tensor_copy(t[:, qb * 512 : (qb + 1) * 512], pp[:])

        def chunk_closures(t, which, hp, qb, min_slot, c0=0, c1=512):
            """(min_slot, closure) items: one per matmul; last also
            evacuates. c0:c1 select columns within the 512-wide chunk."""
            pstate = {}
            w = c1 - c0

            def mk(dc):
                def f():
                    if dc == 0:
                        pstate["pp"] = ps_m.tile([P, 512], F32,
                                                 tag="proj", name="pp")
                    nc.tensor.matmul(
                        pstate["pp"][:, 0:w],
                        ws[which][:, dc, hp * P : (hp + 1) * P],
                        xs[which][:, dc, qb * 512 + c0 : qb * 512 + c1],
                        start=(dc == 0),
                        stop=(dc == DC - 1),
                    )
                    if dc == DC - 1:
                        nc.vector.tensor_copy(
                            t[:, qb * 512 + c0 : qb * 512 + c1],
                            pstate["pp"][:, 0:w])
                return f

            return [(min_slot, mk(d)) for d in range(DC)]

        def drain(q, budget, gate=None):
            while budget and q:
                head = q[0]
                if head[0] is not None and head[0] > slot[0]:
                    break
                if gate is not None and not gate(head):
                    break
                q.pop(0)[1]()
                budget -= 1

        def attn_block(hp, qb, qt, kt, proj_q, v_q, av_q, kt_done, q0_q, first_hp=False, last=False):
            # o banks: [128(q), 2(qc half), 2(head), 66]; qc 0,1 -> bank A,
            # qc 2,3 -> bank B
            o_ps = [ps_o.tile([P, 2, 2, 66], F32, tag=f"O{i}", name=f"o{i}")
                    for i in (0, 1)]

            def emit_scores(kc):
                s = ps_s.tile([P, 2, 512], F32, tag="S", name="s")
                for h in (0, 1):
                    nc.tensor.matmul(
                        s[:, h, :],
                        kt[h * DH : (h + 1) * DH, kc * P : (kc + 1) * P],
                        qt[h * DH : (h + 1) * DH, qb * 512 : (qb + 1) * 512],
                        start=True,
                        stop=True,
                    )
                return s

            def mk_av(kc, pt):
                def f():
                    for qc in range(4):
                        for h in (0, 1):
                            nc.tensor.matmul(
                                o_ps[qc // 2][:, qc % 2, h, :],
                                pt[:, h, qc * P : (qc + 1) * P],
                                vta[:, kc, hp * 2 + h, :],
                                start=(kc == 0 and qc % 2 == 0 and h == 0),
                                stop=(kc == KC - 1),
                                skip_group_check=(qc + h > 0),
                            )
                return f

            def finalize():
                ot = otpool.tile([P, 4, 2, DH], F32, tag="ot", name="ot")
                for i in (0, 1):
                    osb = opool.tile([P, 2, 2, 66], F32, tag="osb", name="osb")
                    nc.vector.tensor_copy(osb[:], o_ps[i][:])
                    for j in (0, 1):
                        for h in (0, 1):
                            rt = rpool.tile([P, 1], F32, tag="rt", name="rt")
                            nc.vector.reciprocal(rt[:], osb[:, j, h, 64:65])
                            nc.vector.tensor_scalar_mul(
                                ot[:, 2 * i + j, h, :], osb[:, j, h, 0:DH],
                                rt[:, 0:1])
                for h in (0, 1):
                    ch = hp * 2 + h
                    nc.sync.dma_start(out[ch, :, qb, :, :], ot[:, :, h, :])

            def gate_av(head):
                kc = head[2]
                return kc is None or vta_done[kc]

            s_cur = emit_scores(0)
            for kc in range(KC):
                pt = ppool.tile([P, 2, 512], F16, tag="p", name="pt")
                nc.scalar.activation(
                    pt[:], s_cur[:],
                    mybir.ActivationFunctionType.Exp,
                    scale=ACT_SCALE)
                slot[0] += 1
                if kc + 1 < KC:
                    if first_hp and qb == 0:
                        # force-drain deferred kt work until the columns the
                        # next scores matmul reads have been projected
                        while kt_done[0] < (kc + 2) * P and proj_q:
                            assert proj_q[0][2] == -1
                            proj_q.pop(0)[1]()
                    s_cur = emit_scores(kc + 1)
                drain(v_q, V_RATE)
                av_q.append((None, mk_av(kc, pt), kc))
                drain(av_q, len(av_q) if last else AV_RATE, gate=gate_av)
                drain(q0_q, 2)
                drain(proj_q, pops)
            av_q.append((None, finalize, None))

        def emit_body():
            emit_loads()
            qt = new_qk("q")
            kt = new_qk("k")
            # warm the PE p-state during the input-DMA wait: dummy
            # matmuls over the zeroed vta keep the ramp model at full speed
            # for the first real projections
            oc = vta[:, 0, :, 64:66]   # [P, HPC, 2] initialized slice
            for i in range(50):
                wp = ps_m.tile([P, 512], F32, tag="proj", name="wp")
                nc.tensor.matmul(
                    wp[0:1, 0:256],
                    oc[:, 0, 0:1],
                    oc.to_broadcast((P, HPC, 2, 16)),
                    start=True, stop=True)
            # prologue: just enough projection for the first scores:
            # Q chunk 0 (xq0 lands first), then K chunk 0 cols 0:256.
            proj_qk_chunk(qt, "q", 0, 0)
            for _s, f in chunk_closures(kt, "k", 0, 0, 0, 0, 256):
                f()
            # deferred, force-drained ahead of the scores that read them
            # (tag -1 entries carry kt columns; kt_cols tracks progress)
            proj_q = []
            kt_done = [256]

            def mark(cols):
                def g():
                    kt_done[0] = cols
                return g

            proj_q += [(0, f, -1) for _s, f in
                       chunk_closures(kt, "k", 0, 0, 0, 256, 512)]
            proj_q.append((0, mark(512), -1))
            proj_q += [(0, f, -1) for _s, f in
                       chunk_closures(kt, "k", 0, 1, 0)]
            proj_q.append((0, mark(1024), -1))
            proj_q += [(4, f, -1) for _s, f in
                       chunk_closures(kt, "k", 0, 2, 4)]
            proj_q.append((4, mark(1536), -1))
            proj_q += [(8, f, -1) for _s, f in
                       chunk_closures(kt, "k", 0, 3, 8)]
            proj_q.append((8, mark(2048), -1))
            q0_q = []
            for qb, ms in ((1, 15), (2, 29), (3, 32)):
                q0_q += [(s0, f, qb) for s0, f in
                         chunk_closures(qt, "q", 0, qb, ms)]
            v_q = []
            for kc in range(KC):
                v_q += v_closures(kc, V_SLOTS[kc // 4] + 2 * (kc % 4))
            av_q = []

            # prefetch queues for hp 1..3, tagged with their hp so the
            # boundary flush can force-complete exactly what's needed
            qts = {0: (qt, kt)}
            for hpn in range(1, HP):
                base = (27, 56, 104, 170)[hpn]
                qts[hpn] = (new_qk("q"), new_qk("k"))
                for qb in range(NQ):
                    proj_q += [(max(s0, base), f, hpn) for s0, f in
                               chunk_closures(qts[hpn][1], "k", hpn, qb, 0)]
                for qb in range(NQ):
                    proj_q += [(max(s0, base, 26), f, hpn) for s0, f in
                               chunk_closures(qts[hpn][0], "q", hpn, qb, 0)]
            for hp in range(HP):
                qt, kt = qts[hp]
                for qb in range(NQ):
                    if hp == 0:
                        # hp0's qt chunk qb must be fully projected before
                        # this block's scores read it
                        while q0_q and q0_q[0][2] <= qb:
                            q0_q.pop(0)[1]()
                    attn_block(hp, qb, qt, kt, proj_q, v_q, av_q, kt_done,
                               q0_q if hp == 0 else [],
                               first_hp=(hp == 0),
                               last=(hp == HP - 1 and qb == NQ - 1))
                # next head pair's projections must be fully emitted before
                # its attention reads them
                if hp + 1 < HP:
                    while proj_q and proj_q[0][2] <= hp + 1:
                        proj_q.pop(0)[1]()
            while v_q:
                v_q.pop(0)[1]()
            while av_q:
                av_q.pop(0)[1]()

        if loop_n > 1:
            with tc.For_i(0, loop_n, 1):
                emit_body()
        else:
            emit_body()

    nc.compile()
    return nc


_NC_CACHE = {}


def _get_nc(S, D, HPC):
    key = (S, D, HPC)
    if key not in _NC_CACHE:
        _NC_CACHE[key] = build_attention(S, D, HPC)
    return _NC_CACHE[key]


def _prep_batch_x(q_seq, k_seq, v_seq, b, D):
    """Per-batch fp16 x^T shards (shared by the 2 head-group cores)."""
    DC = D // P

    def xt(x):  # [S, D] -> [P, DC, S]
        return np.ascontiguousarray(
            x.T.reshape(DC, P, -1).transpose(1, 0, 2)).astype(np.float16)

    return {"xq": xt(q_seq[b]), "xk": xt(k_seq[b]), "xv": xt(v_seq[b])}


def _prep_w(WQ, WK, WV, hg, HPC, D):
    """Per-head-group fp16 weight shards."""
    DC = D // P
    CW = HPC * DH

    def wslice(w):  # [D, out] -> [P, DC, CW]
        return np.ascontiguousarray(
            w[:, hg * CW : (hg + 1) * CW]
            .reshape(DC, P, CW).transpose(1, 0, 2)).astype(np.float16)

    return {"wq": wslice(WQ), "wk": wslice(WK), "wv": wslice(WV)}


def _prep_core_inputs(q_seq, k_seq, v_seq, WQ, WK, WV, b, hg, HPC, D):
    """Host-side shard prep for core (batch b, head group hg)."""
    m = _prep_batch_x(q_seq, k_seq, v_seq, b, D)
    m.update(_prep_w(WQ, WK, WV, hg, HPC, D))
    return m


def kernel(q_seq, k_seq, v_seq, WQ, WK, WV, _trace=False):
    q_seq = np.asarray(q_seq, dtype=np.float32)
    k_seq = np.asarray(k_seq, dtype=np.float32)
    v_seq = np.asarray(v_seq, dtype=np.float32)
    WQ = np.asarray(WQ, dtype=np.float32)
    WK = np.asarray(WK, dtype=np.float32)
    WV = np.asarray(WV, dtype=np.float32)

    B, S, D = q_seq.shape
    NB_HEAD = WQ.shape[1] // DH
    n_cores = 8
    groups_per_batch = n_cores // B          # 2 head groups
    HPC = NB_HEAD // groups_per_batch        # 8 heads per core
    CW = HPC * DH

    nc = _get_nc(S, D, HPC)

    xmaps = {b: _prep_batch_x(q_seq, k_seq, v_seq, b, D) for b in range(B)}
    wmaps = {hg: _prep_w(WQ, WK, WV, hg, HPC, D) for hg in range(groups_per_batch)}
    in_maps = []
    for core in range(n_cores):
        b, hg = core // groups_per_batch, core % groups_per_batch
        in_maps.append({**xmaps[b], **wmaps[hg]})

    res = run_bass_kernel_spmd(
        nc, in_maps, core_ids=list(range(n_cores)), trace=_trace,
        **({"trace_cores": [0], } if _trace else {}),
    )
    if _trace:
        print(f"HW exec time: {res.exec_time_ns} ns")
        if res.instructions_and_trace:
            print("trace:", res.instructions_and_trace[1])

    out = np.empty((B, S, NB_HEAD * DH), dtype=np.float32)
    for core in range(n_cores):
        b, hg = core // groups_per_batch, core % groups_per_batch
        # device output: [HPC, P, NQ, 4, DH]; q = qb*512 + qc*128 + p
        ot = res.results[core]["out"]
        ot = ot.transpose(2, 3, 1, 0, 4).reshape(S, CW)
        out[b, :, hg * CW : (hg + 1) * CW] = ot
    return out


# revision 40
# speedup vs baseline: 1.0333x; 1.0333x over previous
"""Multi-head attention Bass kernel for Trainium2, SPMD over 8 NeuronCores.

Problem: B=4, S=2048, D=1024, 16 heads x 64. Sharding: core = (batch b, head-group hg)
with b in 0..3, hg in 0..1 -> each core computes 8 heads of one batch.

Design (cost-model driven, fp16 end-to-end):
  - ScalarE's exp is the hard floor: 256 activations of [128(k), 2(head),
    512(q)] PSUM fp32 -> p fp16, ~1.04us each = ~266us. Everything else is
    arranged to hide under it.
  - scores (PE): per (kc, head) one fp16 matmul K=64 -> s[k, q] in PSUM.
  - AV is Q-MAJOR: O[q, dh] accumulates with M=128 q-positions on PSUM
    partitions and only N=66 columns (64 dh + a ones column that picks up
    the softmax denominator for free + 1 pad for 8B alignment). lhsT is
    the p tile (stationary), rhs is V-augmented [k, 66]. Cost: 66 cycles
    per (kc, head, q-128-chunk) -> ~58us total, half of the k-major form,
    and the denominator lands per-partition-aligned with q so normalize is
    reciprocal + tensor_scalar_mul per chunk - no cross-partition traffic.
  - Four accumulation chains share each PSUM bank; only the chain writing
    first uses start=True (clears the whole bank's has_written bits), the
    others overwrite-where-unset.
  - All deferred work (V projection chunks, AV+finalize, later Q/K
    projections) sits in queues annotated with the earliest "slot" (ACT
    count) at which its input DMA will have landed, so a not-yet-ready
    instruction never enters the PE FIFO ahead of the scores matmuls that
    feed ScalarE. AV closures additionally gate on their V chunk being
    emitted; finalize closures ride the same queue so o_ps frees in order.
  - inputs stream as column-sliced DMAs in consumption order (the DMA
    engine pool is serial in the cost model): wq, wk, xk0, xq0, xk1, xv0,
    xk2, xv1, xk3, xq1, xv2, xv3, xq2, xq3.

PSUM (8 banks): scores 2bufs x [128,2,512] f32 = 4, O accum 2 (2qc x 2h x 66
x 2 banks), projection staging 2.
"""
import numpy as np
import ml_dtypes
from contextlib import ExitStack

import concourse.tile as tile
import concourse.mybir as mybir
from concourse import bacc
from concourse.bass_utils import run_bass_kernel_spmd

P = 128
DH = 64
F16 = mybir.dt.float16
F32 = mybir.dt.float32
FP8 = mybir.dt.float8e4
DR = mybir.MatmulPerfMode.DoubleRow
XS, WSC = 4.0, 64.0

AV_START = 2          # earliest kc for AV draining (first block)
AV_RATE = 6           # max AV/finalize closures drained per kc
V_RATE = 6            # max V-projection closures drained per kc
# earliest global slot for V chunk group c//4 (when xv quarter c//4 landed)
V_SLOTS = (13, 16, 19, 22)
PQ_RATE = 5           # max projection closures drained per kc


def build_attention(S=2048, D=1024, HPC=8, loop_n=1, pops=PQ_RATE):
    """Build the per-core SPMD program. HPC = heads per core (even).

    loop_n > 1 wraps the whole body in a hardware loop (for timing)."""
    DC = D // P        # D chunks of 128
    KC = S // P        # k chunks of 128
    NQ = S // 512      # q blocks of 512
    HP = HPC // 2      # head pairs
    CW = HPC * DH      # core output width
    ACT_SCALE = 1.0 / float(np.sqrt(DH)) / (XS * XS * WSC * WSC)
    DC2 = DC // 2

    nc = bacc.Bacc("TRN2")
    xd, wd = {}, {}
    for n in ("q", "k", "v"):
        xd[n] = nc.dram_tensor("x" + n, [P, 2, DC, S], FP8,
                               kind="ExternalInput")
        wd[n] = nc.dram_tensor("w" + n, [P, 2, DC, CW], FP8,
                               kind="ExternalInput")
    out = nc.dram_tensor("out", [HPC, P, NQ, 4, DH], F32, kind="ExternalOutput")

    with tile.TileContext(nc) as tc, ExitStack() as ctx:
        xpool = ctx.enter_context(tc.tile_pool(name="x", bufs=1))
        wpool = ctx.enter_context(tc.tile_pool(name="w", bufs=1))
        vpool = ctx.enter_context(tc.tile_pool(name="v", bufs=1))
        qkpool = ctx.enter_context(tc.tile_pool(name="qk", bufs=3))
        ppool = ctx.enter_context(tc.tile_pool(name="p", bufs=20))
        rpool = ctx.enter_context(tc.tile_pool(name="r", bufs=4))
        opool = ctx.enter_context(tc.tile_pool(name="ob", bufs=2))
        otpool = ctx.enter_context(tc.tile_pool(name="ot", bufs=2))
        ps_s = ctx.enter_context(tc.tile_pool(name="ps_s", bufs=2, space="PSUM"))
        ps_o = ctx.enter_context(tc.tile_pool(name="ps_o", bufs=1, space="PSUM"))
        ps_m = ctx.enter_context(tc.tile_pool(name="ps_m", bufs=2, space="PSUM"))

        xs, ws = {}, {}
        vta = None
        slot = [0]           # global ACT counter
        vta_done = [False] * KC

        def emit_loads():
            nonlocal vta
            for name in ("q", "k", "v"):
                ws[name] = wpool.tile([P, 2, DC, CW], FP8, tag="w" + name,
                                      name="w" + name)
                xs[name] = xpool.tile([P, 2, DC, S], FP8, tag="x" + name,
                                      name="x" + name)

            def ld(n, c0, c1):
                nc.sync.dma_start(xs[n][:, :, :, c0:c1],
                                  xd[n][:, :, :, c0:c1])

            def ldw(n, c0, c1):
                nc.sync.dma_start(ws[n][:, :, :, c0:c1],
                                  wd[n][:, :, :, c0:c1])

            # DMA order = consumption order (DMA engine pool is serial).
            # hp0's weight columns first; the rest of W after the k/v bulk.
            ldw("q", 0, P)
            ldw("k", 0, P)
            ld("q", 0, 512)
            ld("k", 0, 512)
            ld("k", 512, 1024)
            ld("k", 1024, 1536)
            ldw("v", 0, CW)
            ld("k", 1536, 2048)
            ld("v", 0, 512)
            ld("q", 512, 1024)
            ld("v", 512, 1024)
            ld("v", 1024, 1536)
            ld("v", 1536, 2048)
            ldw("q", P, CW)
            ldw("k", P, CW)
            ld("q", 1024, 1536)
            ld("q", 1536, 2048)
            # V-augmented rhs: [kpos, kc, ch, 66] = V | 1.0 | 0 pad
            vta = vpool.tile([P, KC, HPC, 66], F16, tag="V", name="vta")
            nc.vector.memset(vta[:, :, :, 64], XS * XS * WSC * WSC / 256.0)
            nc.vector.memset(vta[:, :, :, 65], 0.0)

        CROSS = ((0, 0), (0, 1), (1, 0))

        def v_closures(kc, min_slot):
            pstate = {}
            n12 = 3 * DC2

            def mk(i):
                ci, dc2 = divmod(i, DC2)
                a, b = CROSS[ci]

                def f():
                    if i == 0:
                        pstate["pv"] = ps_m.tile([P, 512], F32,
                                                 tag="proj", name="pv")
                    nc.tensor.matmul(
                        pstate["pv"][:, :CW],
                        xs["v"][:, a, 2 * dc2 : 2 * dc2 + 2,
                                kc * P : (kc + 1) * P],
                        ws["v"][:, b, 2 * dc2 : 2 * dc2 + 2, :],
                        start=(i == 0),
                        stop=(i == n12 - 1),
                        perf_mode=DR,
                    )
                    if i == n12 - 1:
                        nc.vector.tensor_copy(
                            vta[:, kc, :, 0:DH],
                            pstate["pv"][:, :CW].rearrange(
                                "p (h d) -> p h d", d=DH),
                        )
                        vta_done[kc] = True
                return f

            return [(min_slot, mk(i)) for i in range(n12)]

        def new_qk(which):
            return qkpool.tile([P, S], F16, tag=which, name=which + "t")

        def proj_qk_chunk(t, which, hp, qb):
            pp = ps_m.tile([P, 512], F32, tag="proj", name="pp")
            n12 = 3 * DC2
            for i in range(n12):
                ci, dc2 = divmod(i, DC2)
                a, b = CROSS[ci]
                nc.tensor.matmul(
                    pp[:],
                    ws[which][:, b, 2 * dc2 : 2 * dc2 + 2,
                              hp * P : (hp + 1) * P],
                    xs[which][:, a, 2 * dc2 : 2 * dc2 + 2,
                              qb * 512 : (qb + 1) * 512],
                    start=(i == 0),
                    stop=(i == n12 - 1),
                    perf_mode=DR,
                )
            nc.vector.tensor_copy(t[:, qb * 512 : (qb + 1) * 512], pp[:])

        def chunk_closures(t, which, hp, qb, min_slot, c0=0, c1=512):
            """(min_slot, closure) items: one per matmul; last also
            evacuates. c0:c1 select columns within the 512-wide chunk."""
            pstate = {}
            w = c1 - c0
            n12 = 3 * DC2

            def mk(i):
                ci, dc2 = divmod(i, DC2)
                a, b = CROSS[ci]

                def f():
                    if i == 0:
                        pstate["pp"] = ps_m.tile([P, 512], F32,
                                                 tag="proj", name="pp")
                    nc.tensor.matmul(
                        pstate["pp"][:, 0:w],
                        ws[which][:, b, 2 * dc2 : 2 * dc2 + 2,
                                  hp * P : (hp + 1) * P],
                        xs[which][:, a, 2 * dc2 : 2 * dc2 + 2,
                                  qb * 512 + c0 : qb * 512 + c1],
                        start=(i == 0),
                        stop=(i == n12 - 1),
                        perf_mode=DR,
                    )
                    if i == n12 - 1:
                        nc.vector.tensor_copy(
                            t[:, qb * 512 + c0 : qb * 512 + c1],
                            pstate["pp"][:, 0:w])
                return f

            return [(min_slot, mk(i)) for i in range(n12)]

        def drain(q, budget, gate=None):
            while budget and q:
                head = q[0]
                if head[0] is not None and head[0] > slot[0]:
                    break
                if gate is not None and not gate(head):
                    break
                q.pop(0)[1]()
                budget -= 1

        def attn_block(hp, qb, qt, kt, proj_q, v_q, av_q, kt_done, q0_q, first_hp=False, last=False):
            # o banks: [128(q), 2(qc half), 2(head), 66]; qc 0,1 -> bank A,
            # qc 2,3 -> bank B
            o_ps = [ps_o.tile([P, 2, 2, 66], F32, tag=f"O{i}", name=f"o{i}")
                    for i in (0, 1)]

            def emit_scores(kc):
                s = ps_s.tile([P, 2, 512], F32, tag="S", name="s")
                for h in (0, 1):
                    nc.tensor.matmul(
                        s[:, h, :],
                        kt[h * DH : (h + 1) * DH, kc * P : (kc + 1) * P],
                        qt[h * DH : (h + 1) * DH, qb * 512 : (qb + 1) * 512],
                        start=True,
                        stop=True,
                    )
                return s

            def mk_av(kc, pt):
                def f():
                    for qc in range(4):
                        for h in (0, 1):
                            nc.tensor.matmul(
                                o_ps[qc // 2][:, qc % 2, h, :],
                                pt[:, h, qc * P : (qc + 1) * P],
                                vta[:, kc, hp * 2 + h, :],
                                start=(kc == 0 and qc % 2 == 0 and h == 0),
                                stop=(kc == KC - 1),
                                skip_group_check=(qc + h > 0),
                            )
                return f

            def finalize():
                ot = otpool.tile([P, 4, 2, DH], F32, tag="ot", name="ot")
                for i in (0, 1):
                    osb = opool.tile([P, 2, 2, 66], F32, tag="osb", name="osb")
                    nc.vector.tensor_copy(osb[:], o_ps[i][:])
                    for j in (0, 1):
                        for h in (0, 1):
                            rt = rpool.tile([P, 1], F32, tag="rt", name="rt")
                            nc.vector.reciprocal(rt[:], osb[:, j, h, 64:65])
                            nc.vector.tensor_scalar_mul(
                                ot[:, 2 * i + j, h, :], osb[:, j, h, 0:DH],
                                rt[:, 0:1])
                for h in (0, 1):
                    ch = hp * 2 + h
                    nc.sync.dma_start(out[ch, :, qb, :, :], ot[:, :, h, :])

            def gate_av(head):
                kc = head[2]
                return kc is None or vta_done[kc]

            s_cur = emit_scores(0)
            for kc in range(KC):
                pt = ppool.tile([P, 2, 512], F16, tag="p", name="pt")
                nc.scalar.activation(
                    pt[:], s_cur[:],
                    mybir.ActivationFunctionType.Exp,
                    scale=ACT_SCALE)
                slot[0] += 1
                if kc + 1 < KC:
                    if first_hp and qb == 0:
                        # force-drain deferred kt work until the columns the
                        # next scores matmul reads have been projected
                        while kt_done[0] < (kc + 2) * P and proj_q:
                            assert proj_q[0][2] == -1
                            proj_q.pop(0)[1]()
                    s_cur = emit_scores(kc + 1)
                drain(v_q, V_RATE)
                av_q.append((None, mk_av(kc, pt), kc))
                drain(av_q, len(av_q) if last else AV_RATE, gate=gate_av)
                drain(q0_q, 2)
                drain(proj_q, pops)
            av_q.append((None, finalize, None))

        def emit_body():
            emit_loads()
            qt = new_qk("q")
            kt = new_qk("k")
            # warm the PE p-state during the input-DMA wait: dummy
            # matmuls over the zeroed vta keep the ramp model at full speed
            # for the first real projections
            oc = vta[:, 0, :, 64:66]   # [P, HPC, 2] initialized slice
            for i in range(50):
                wp = ps_m.tile([P, 512], F32, tag="proj", name="wp")
                nc.tensor.matmul(
                    wp[0:1, 0:256],
                    oc[:, 0, 0:1],
                    oc.to_broadcast((P, HPC, 2, 16)),
                    start=True, stop=True)
            # prologue: just enough projection for the first scores:
            # Q chunk 0 (xq0 lands first), then K chunk 0 cols 0:256.
            proj_qk_chunk(qt, "q", 0, 0)
            for _s, f in chunk_closures(kt, "k", 0, 0, 0, 0, 256):
                f()
            # deferred, force-drained ahead of the scores that read them
            # (tag -1 entries carry kt columns; kt_cols tracks progress)
            proj_q = []
            kt_done = [256]

            def mark(cols):
                def g():
                    kt_done[0] = cols
                return g

            proj_q += [(0, f, -1) for _s, f in
                       chunk_closures(kt, "k", 0, 0, 0, 256, 512)]
            proj_q.append((0, mark(512), -1))
            proj_q += [(0, f, -1) for _s, f in
                       chunk_closures(kt, "k", 0, 1, 0)]
            proj_q.append((0, mark(1024), -1))
            proj_q += [(4, f, -1) for _s, f in
                       chunk_closures(kt, "k", 0, 2, 4)]
            proj_q.append((4, mark(1536), -1))
            proj_q += [(8, f, -1) for _s, f in
                       chunk_closures(kt, "k", 0, 3, 8)]
            proj_q.append((8, mark(2048), -1))
            q0_q = []
            for qb, ms in ((1, 15), (2, 29), (3, 32)):
                q0_q += [(s0, f, qb) for s0, f in
                         chunk_closures(qt, "q", 0, qb, ms)]
            v_q = []
            for kc in range(KC):
                v_q += v_closures(kc, V_SLOTS[kc // 4] + 2 * (kc % 4))
            av_q = []

            # prefetch queues for hp 1..3, tagged with their hp so the
            # boundary flush can force-complete exactly what's needed
            qts = {0: (qt, kt)}
            for hpn in range(1, HP):
                base = (27, 56, 104, 170)[hpn]
                qts[hpn] = (new_qk("q"), new_qk("k"))
                for qb in range(NQ):
                    proj_q += [(max(s0, base), f, hpn) for s0, f in
                               chunk_closures(qts[hpn][1], "k", hpn, qb, 0)]
                for qb in range(NQ):
                    proj_q += [(max(s0, base, 26), f, hpn) for s0, f in
                               chunk_closures(qts[hpn][0], "q", hpn, qb, 0)]
            for hp in range(HP):
                qt, kt = qts[hp]
                for qb in range(NQ):
                    if hp == 0:
                        # hp0's qt chunk qb must be fully projected before
                        # this block's scores read it
                        while q0_q and q0_q[0][2] <= qb:
                            q0_q.pop(0)[1]()
                    attn_block(hp, qb, qt, kt, proj_q, v_q, av_q, kt_done,
                               q0_q if hp == 0 else [],
                               first_hp=(hp == 0),
                               last=(hp == HP - 1 and qb == NQ - 1))
                # next head pair's projections must be fully emitted before
                # its attention reads them
                if hp + 1 < HP:
                    while proj_q and proj_q[0][2] <= hp + 1:
                        proj_q.pop(0)[1]()
            while v_q:
                v_q.pop(0)[1]()
            while av_q:
                av_q.pop(0)[1]()

        if loop_n > 1:
            with tc.For_i(0, loop_n, 1):
                emit_body()
        else:
            emit_body()

    nc.compile()
    return nc


_NC_CACHE = {}


def _get_nc(S, D, HPC):
    key = (S, D, HPC)
    if key not in _NC_CACHE:
        _NC_CACHE[key] = build_attention(S, D, HPC)
    return _NC_CACHE[key]


def _prep_batch_x(q_seq, k_seq, v_seq, b, D):
    """Per-batch fp16 x^T shards (shared by the 2 head-group cores)."""
    DC = D // P

    e4m3 = ml_dtypes.float8_e4m3

    def xt(x):  # [S, D] -> [P, 2, DC, S]; 4x split into fp8 hi+lo
        t = np.ascontiguousarray(
            (XS * x).T.reshape(DC, P, -1).transpose(1, 0, 2),
            dtype=np.float32)
        hi = t.astype(e4m3)
        lo = (t - hi.astype(np.float32)).astype(e4m3)
        return np.ascontiguousarray(np.stack([hi, lo], axis=1))

    return {"xq": xt(q_seq[b]), "xk": xt(k_seq[b]), "xv": xt(v_seq[b])}


def _prep_w(WQ, WK, WV, hg, HPC, D):
    """Per-head-group fp16 weight shards."""
    DC = D // P
    CW = HPC * DH

    e4m3 = ml_dtypes.float8_e4m3

    def wslice(w):  # [D, out] -> [P, 2, DC, CW]; 64x split into fp8 hi+lo
        t = np.ascontiguousarray(
            (WSC * w[:, hg * CW : (hg + 1) * CW])
            .reshape(DC, P, CW).transpose(1, 0, 2), dtype=np.float32)
        hi = t.astype(e4m3)
        lo = (t - hi.astype(np.float32)).astype(e4m3)
        return np.ascontiguousarray(np.stack([hi, lo], axis=1))

    return {"wq": wslice(WQ), "wk": wslice(WK), "wv": wslice(WV)}


def _prep_core_inputs(q_seq, k_seq, v_seq, WQ, WK, WV, b, hg, HPC, D):
    """Host-side shard prep for core (batch b, head group hg)."""
    m = _prep_batch_x(q_seq, k_seq, v_seq, b, D)
    m.update(_prep_w(WQ, WK, WV, hg, HPC, D))
    return m


def kernel(q_seq, k_seq, v_seq, WQ, WK, WV, _trace=False):
    q_seq = np.asarray(q_seq, dtype=np.float32)
    k_seq = np.asarray(k_seq, dtype=np.float32)
    v_seq = np.asarray(v_seq, dtype=np.float32)
    WQ = np.asarray(WQ, dtype=np.float32)
    WK = np.asarray(WK, dtype=np.float32)
    WV = np.asarray(WV, dtype=np.float32)

    B, S, D = q_seq.shape
    NB_HEAD = WQ.shape[1] // DH
    n_cores = 8
    groups_per_batch = n_cores // B          # 2 head groups
    HPC = NB_HEAD // groups_per_batch        # 8 heads per core
    CW = HPC * DH

    nc = _get_nc(S, D, HPC)

    xmaps = {b: _prep_batch_x(q_seq, k_seq, v_seq, b, D) for b in range(B)}
    wmaps = {hg: _prep_w(WQ, WK, WV, hg, HPC, D) for hg in range(groups_per_batch)}
    in_maps = []
    for core in range(n_cores):
        b, hg = core // groups_per_batch, core % groups_per_batch
        in_maps.append({**xmaps[b], **wmaps[hg]})

    res = run_bass_kernel_spmd(
        nc, in_maps, core_ids=list(range(n_cores)), trace=_trace,
        **({"trace_cores": [0], } if _trace else {}),
    )
    if _trace:
        print(f"HW exec time: {res.exec_time_ns} ns")
        if res.instructions_and_trace:
            print("trace:", res.instructions_and_trace[1])

    out = np.empty((B, S, NB_HEAD * DH), dtype=np.float32)
    for core in range(n_cores):
        b, hg = core // groups_per_batch, core % groups_per_batch
        # device output: [HPC, P, NQ, 4, DH]; q = qb*512 + qc*128 + p
        ot = res.results[core]["out"]
        ot = ot.transpose(2, 3, 1, 0, 4).reshape(S, CW)
        out[b, :, hg * CW : (hg + 1) * CW] = ot
    return out


# revision 43
# speedup vs baseline: 1.0402x; 1.0067x over previous
"""Multi-head attention Bass kernel for Trainium2, SPMD over 8 NeuronCores.

Problem: B=4, S=2048, D=1024, 16 heads x 64. Sharding: core = (batch b, head-group hg)
with b in 0..3, hg in 0..1 -> each core computes 8 heads of one batch.

Design (cost-model driven, fp16 end-to-end):
  - ScalarE's exp is the hard floor: 256 activations of [128(k), 2(head),
    512(q)] PSUM fp32 -> p fp16, ~1.04us each = ~266us. Everything else is
    arranged to hide under it.
  - scores (PE): per (kc, head) one fp16 matmul K=64 -> s[k, q] in PSUM.
  - AV is Q-MAJOR: O[q, dh] accumulates with M=128 q-positions on PSUM
    partitions and only N=66 columns (64 dh + a ones column that picks up
    the softmax denominator for free + 1 pad for 8B alignment). lhsT is
    the p tile (stationary), rhs is V-augmented [k, 66]. Cost: 66 cycles
    per (kc, head, q-128-chunk) -> ~58us total, half of the k-major form,
    and the denominator lands per-partition-aligned with q so normalize is
    reciprocal + tensor_scalar_mul per chunk - no cross-partition traffic.
  - Four accumulation chains share each PSUM bank; only the chain writing
    first uses start=True (clears the whole bank's has_written bits), the
    others overwrite-where-unset.
  - All deferred work (V projection chunks, AV+finalize, later Q/K
    projections) sits in queues annotated with the earliest "slot" (ACT
    count) at which its input DMA will have landed, so a not-yet-ready
    instruction never enters the PE FIFO ahead of the scores matmuls that
    feed ScalarE. AV closures additionally gate on their V chunk being
    emitted; finalize closures ride the same queue so o_ps frees in order.
  - inputs stream as column-sliced DMAs in consumption order (the DMA
    engine pool is serial in the cost model): wq, wk, xk0, xq0, xk1, xv0,
    xk2, xv1, xk3, xq1, xv2, xv3, xq2, xq3.

PSUM (8 banks): scores 2bufs x [128,2,512] f32 = 4, O accum 2 (2qc x 2h x 66
x 2 banks), projection staging 2.
"""
import numpy as np
import ml_dtypes
from contextlib import ExitStack

import concourse.tile as tile
import concourse.mybir as mybir
from concourse import bacc
from concourse.bass_utils import run_bass_kernel_spmd

P = 128
DH = 64
F16 = mybir.dt.float16
F32 = mybir.dt.float32
FP8 = mybir.dt.float8e4
DR = mybir.MatmulPerfMode.DoubleRow
XS, WSC = 4.0, 64.0

AV_START = 2          # earliest kc for AV draining (first block)
AV_RATE = 6           # max AV/finalize closures drained per kc
V_RATE = 6            # max V-projection closures drained per kc
# earliest global slot for V chunk group c//4 (when xv quarter c//4 landed)
V_SLOTS = (10, 13, 16, 19)
PQ_RATE = 5           # max projection closures drained per kc


def build_attention(S=2048, D=1024, HPC=8, loop_n=1, pops=PQ_RATE):
    """Build the per-core SPMD program. HPC = heads per core (even).

    loop_n > 1 wraps the whole body in a hardware loop (for timing)."""
    DC = D // P        # D chunks of 128
    KC = S // P        # k chunks of 128
    NQ = S // 512      # q blocks of 512
    HP = HPC // 2      # head pairs
    CW = HPC * DH      # core output width
    ACT_SCALE = 1.0 / float(np.sqrt(DH)) / (XS * XS * WSC * WSC)
    DC2 = DC // 2

    nc = bacc.Bacc("TRN2")
    xd, wd = {}, {}
    for n in ("q", "k", "v"):
        xd[n] = nc.dram_tensor("x" + n, [P, 2, DC, S], FP8,
                               kind="ExternalInput")
        wd[n] = nc.dram_tensor("w" + n, [P, 2, DC, CW], FP8,
                               kind="ExternalInput")
    out = nc.dram_tensor("out", [HPC, P, NQ, 4, DH], F32, kind="ExternalOutput")

    with tile.TileContext(nc) as tc, ExitStack() as ctx:
        xpool = ctx.enter_context(tc.tile_pool(name="x", bufs=1))
        wpool = ctx.enter_context(tc.tile_pool(name="w", bufs=1))
        vpool = ctx.enter_context(tc.tile_pool(name="v", bufs=1))
        qkpool = ctx.enter_context(tc.tile_pool(name="qk", bufs=3))
        ppool = ctx.enter_context(tc.tile_pool(name="p", bufs=20))
        rpool = ctx.enter_context(tc.tile_pool(name="r", bufs=4))
        opool = ctx.enter_context(tc.tile_pool(name="ob", bufs=2))
        otpool = ctx.enter_context(tc.tile_pool(name="ot", bufs=2))
        ps_s = ctx.enter_context(tc.tile_pool(name="ps_s", bufs=2, space="PSUM"))
        ps_o = ctx.enter_context(tc.tile_pool(name="ps_o", bufs=1, space="PSUM"))
        ps_m = ctx.enter_context(tc.tile_pool(name="ps_m", bufs=2, space="PSUM"))

        xs, ws = {}, {}
        vta = None
        slot = [0]           # global ACT counter
        vta_done = [False] * KC

        def emit_loads():
            nonlocal vta
            for name in ("q", "k", "v"):
                ws[name] = wpool.tile([P, 2, DC, CW], FP8, tag="w" + name,
                                      name="w" + name)
                xs[name] = xpool.tile([P, 2, DC, S], FP8, tag="x" + name,
                                      name="x" + name)

            def ld(n, c0, c1):
                nc.sync.dma_start(xs[n][:, :, :, c0:c1],
                                  xd[n][:, :, :, c0:c1])

            def ldw(n, c0, c1):
                nc.sync.dma_start(ws[n][:, :, :, c0:c1],
                                  wd[n][:, :, :, c0:c1])

            # DMA order = consumption order (DMA engine pool is serial).
            # hp0's weight columns first; the rest of W after the k/v bulk.
            ldw("q", 0, P)
            ldw("k", 0, P)
            ld("q", 0, 512)
            ld("k", 0, 512)
            ld("k", 512, 1024)
            ld("k", 1024, 1536)
            ldw("v", 0, CW)
            ld("k", 1536, 2048)
            ld("v", 0, 512)
            ld("q", 512, 1024)
            ld("v", 512, 1024)
            ld("v", 1024, 1536)
            ld("v", 1536, 2048)
            ldw("q", P, CW)
            ldw("k", P, CW)
            ld("q", 1024, 1536)
            ld("q", 1536, 2048)
            # V-augmented rhs: [kpos, kc, ch, 66] = V | 1.0 | 0 pad
            vta = vpool.tile([P, KC, HPC, 66], F16, tag="V", name="vta")
            nc.vector.memset(vta[:, :, :, 64], XS * XS * WSC * WSC / 256.0)
            nc.vector.memset(vta[:, :, :, 65], 0.0)

        CROSS = ((0, 0), (0, 1), (1, 0))

        def v_closures(kc, min_slot):
            pstate = {}
            n12 = 3 * DC2

            def mk(i):
                ci, dc2 = divmod(i, DC2)
                a, b = CROSS[ci]

                def f():
                    if i == 0:
                        pstate["pv"] = ps_m.tile([P, 512], F32,
                                                 tag="proj", name="pv")
                    nc.tensor.matmul(
                        pstate["pv"][:, :CW],
                        xs["v"][:, a, 2 * dc2 : 2 * dc2 + 2,
                                kc * P : (kc + 1) * P],
                        ws["v"][:, b, 2 * dc2 : 2 * dc2 + 2, :],
                        start=(i == 0),
                        stop=(i == n12 - 1),
                        perf_mode=DR,
                    )
                    if i == n12 - 1:
                        nc.vector.tensor_copy(
                            vta[:, kc, :, 0:DH],
                            pstate["pv"][:, :CW].rearrange(
                                "p (h d) -> p h d", d=DH),
                        )
                        vta_done[kc] = True
                return f

            return [(min_slot, mk(i)) for i in range(n12)]

        def new_qk(which):
            return qkpool.tile([P, S], F16, tag=which, name=which + "t")

        def proj_qk_chunk(t, which, hp, qb):
            pp = ps_m.tile([P, 512], F32, tag="proj", name="pp")
            n12 = 3 * DC2
            for i in range(n12):
                ci, dc2 = divmod(i, DC2)
                a, b = CROSS[ci]
                nc.tensor.matmul(
                    pp[:],
                    ws[which][:, b, 2 * dc2 : 2 * dc2 + 2,
                              hp * P : (hp + 1) * P],
                    xs[which][:, a, 2 * dc2 : 2 * dc2 + 2,
                              qb * 512 : (qb + 1) * 512],
                    start=(i == 0),
                    stop=(i == n12 - 1),
                    perf_mode=DR,
                )
            nc.vector.tensor_copy(t[:, qb * 512 : (qb + 1) * 512], pp[:])

        def chunk_closures(t, which, hp, qb, min_slot, c0=0, c1=512):
            """(min_slot, closure) items: one per matmul; last also
            evacuates. c0:c1 select columns within the 512-wide chunk."""
            pstate = {}
            w = c1 - c0
            n12 = 3 * DC2

            def mk(i):
                ci, dc2 = divmod(i, DC2)
                a, b = CROSS[ci]

                def f():
                    if i == 0:
                        pstate["pp"] = ps_m.tile([P, 512], F32,
                                                 tag="proj", name="pp")
                    nc.tensor.matmul(
                        pstate["pp"][:, 0:w],
                        ws[which][:, b, 2 * dc2 : 2 * dc2 + 2,
                                  hp * P : (hp + 1) * P],
                        xs[which][:, a, 2 * dc2 : 2 * dc2 + 2,
                                  qb * 512 + c0 : qb * 512 + c1],
                        start=(i == 0),
                        stop=(i == n12 - 1),
                        perf_mode=DR,
                    )
                    if i == n12 - 1:
                        nc.vector.tensor_copy(
                            t[:, qb * 512 + c0 : qb * 512 + c1],
                            pstate["pp"][:, 0:w])
                return f

            return [(min_slot, mk(i)) for i in range(n12)]

        def drain(q, budget, gate=None):
            while budget and q:
                head = q[0]
                if head[0] is not None and head[0] > slot[0]:
                    break
                if gate is not None and not gate(head):
                    break
                q.pop(0)[1]()
                budget -= 1

        def attn_block(hp, qb, qt, kt, proj_q, v_q, av_q, kt_done, q0_q, first_hp=False, last=False):
            # o banks: [128(q), 2(qc half), 2(head), 66]; qc 0,1 -> bank A,
            # qc 2,3 -> bank B
            o_ps = [ps_o.tile([P, 2, 2, 66], F32, tag=f"O{i}", name=f"o{i}")
                    for i in (0, 1)]

            def emit_scores(kc):
                s = ps_s.tile([P, 2, 512], F32, tag="S", name="s")
                for h in (0, 1):
                    nc.tensor.matmul(
                        s[:, h, :],
                        kt[h * DH : (h + 1) * DH, kc * P : (kc + 1) * P],
                        qt[h * DH : (h + 1) * DH, qb * 512 : (qb + 1) * 512],
                        start=True,
                        stop=True,
                    )
                return s

            def mk_av(kc, pt):
                def f():
                    for qc in range(4):
                        for h in (0, 1):
                            nc.tensor.matmul(
                                o_ps[qc // 2][:, qc % 2, h, :],
                                pt[:, h, qc * P : (qc + 1) * P],
                                vta[:, kc, hp * 2 + h, :],
                                start=(kc == 0 and qc % 2 == 0 and h == 0),
                                stop=(kc == KC - 1),
                                skip_group_check=(qc + h > 0),
                            )
                return f

            def finalize():
                ot = otpool.tile([P, 4, 2, DH], F32, tag="ot", name="ot")
                for i in (0, 1):
                    osb = opool.tile([P, 2, 2, 66], F32, tag="osb", name="osb")
                    nc.vector.tensor_copy(osb[:], o_ps[i][:])
                    for j in (0, 1):
                        for h in (0, 1):
                            rt = rpool.tile([P, 1], F32, tag="rt", name="rt")
                            nc.vector.reciprocal(rt[:], osb[:, j, h, 64:65])
                            nc.vector.tensor_scalar_mul(
                                ot[:, 2 * i + j, h, :], osb[:, j, h, 0:DH],
                                rt[:, 0:1])
                for h in (0, 1):
                    ch = hp * 2 + h
                    nc.sync.dma_start(out[ch, :, qb, :, :], ot[:, :, h, :])

            def gate_av(head):
                kc = head[2]
                return kc is None or vta_done[kc]

            s_cur = emit_scores(0)
            for kc in range(KC):
                pt = ppool.tile([P, 2, 512], F16, tag="p", name="pt")
                nc.scalar.activation(
                    pt[:], s_cur[:],
                    mybir.ActivationFunctionType.Exp,
                    scale=ACT_SCALE)
                slot[0] += 1
                if kc + 1 < KC:
                    if first_hp and qb == 0:
                        # force-drain deferred kt work until the columns the
                        # next scores matmul reads have been projected
                        while kt_done[0] < (kc + 2) * P and proj_q:
                            assert proj_q[0][2] == -1
                            proj_q.pop(0)[1]()
                    s_cur = emit_scores(kc + 1)
                drain(v_q, V_RATE)
                av_q.append((None, mk_av(kc, pt), kc))
                drain(av_q, len(av_q) if last else AV_RATE, gate=gate_av)
                drain(q0_q, 2)
                drain(proj_q, pops)
            av_q.append((None, finalize, None))

        def emit_body():
            emit_loads()
            qt = new_qk("q")
            kt = new_qk("k")
            # warm the PE p-state during the input-DMA wait: dummy
            # matmuls over the zeroed vta keep the ramp model at full speed
            # for the first real projections
            oc = vta[:, 0, :, 64:66]   # [P, HPC, 2] initialized slice
            for i in range(50):
                wp = ps_m.tile([P, 512], F32, tag="proj", name="wp")
                nc.tensor.matmul(
                    wp[0:1, 0:256],
                    oc[:, 0, 0:1],
                    oc.to_broadcast((P, HPC, 2, 16)),
                    start=True, stop=True)
            # prologue: just enough projection for the first scores:
            # Q chunk 0 (xq0 lands first), then K chunk 0 cols 0:256.
            proj_qk_chunk(qt, "q", 0, 0)
            for _s, f in chunk_closures(kt, "k", 0, 0, 0, 0, 256):
                f()
            # deferred, force-drained ahead of the scores that read them
            # (tag -1 entries carry kt columns; kt_cols tracks progress)
            proj_q = []
            kt_done = [256]

            def mark(cols):
                def g():
                    kt_done[0] = cols
                return g

            proj_q += [(0, f, -1) for _s, f in
                       chunk_closures(kt, "k", 0, 0, 0, 256, 512)]
            proj_q.append((0, mark(512), -1))
            proj_q += [(0, f, -1) for _s, f in
                       chunk_closures(kt, "k", 0, 1, 0)]
            proj_q.append((0, mark(1024), -1))
            proj_q += [(4, f, -1) for _s, f in
                       chunk_closures(kt, "k", 0, 2, 4)]
            proj_q.append((4, mark(1536), -1))
            proj_q += [(8, f, -1) for _s, f in
                       chunk_closures(kt, "k", 0, 3, 8)]
            proj_q.append((8, mark(2048), -1))
            q0_q = []
            for qb, ms in ((1, 15), (2, 29), (3, 32)):
                q0_q += [(s0, f, qb) for s0, f in
                         chunk_closures(qt, "q", 0, qb, ms)]
            v_q = []
            for kc in range(KC):
                v_q += v_closures(kc, V_SLOTS[kc // 4] + 2 * (kc % 4))
            av_q = []

            # prefetch queues for hp 1..3, tagged with their hp so the
            # boundary flush can force-complete exactly what's needed
            qts = {0: (qt, kt)}
            for hpn in range(1, HP):
                base = (27, 56, 104, 170)[hpn]
                qts[hpn] = (new_qk("q"), new_qk("k"))
                for qb in range(NQ):
                    proj_q += [(max(s0, base), f, hpn) for s0, f in
                               chunk_closures(qts[hpn][1], "k", hpn, qb, 0)]
                for qb in range(NQ):
                    proj_q += [(max(s0, base, 26), f, hpn) for s0, f in
                               chunk_closures(qts[hpn][0], "q", hpn, qb, 0)]
            for hp in range(HP):
                qt, kt = qts[hp]
                for qb in range(NQ):
                    if hp == 0:
                        # hp0's qt chunk qb must be fully projected before
                        # this block's scores read it
                        while q0_q and q0_q[0][2] <= qb:
                            q0_q.pop(0)[1]()
                    attn_block(hp, qb, qt, kt, proj_q, v_q, av_q, kt_done,
                               q0_q if hp == 0 else [],
                               first_hp=(hp == 0),
                               last=(hp == HP - 1 and qb == NQ - 1))
                # next head pair's projections must be fully emitted before
                # its attention reads them
                if hp + 1 < HP:
                    while proj_q and proj_q[0][2] <= hp + 1:
                        proj_q.pop(0)[1]()
            while v_q:
                v_q.pop(0)[1]()
            while av_q:
                av_q.pop(0)[1]()

        if loop_n > 1:
            with tc.For_i(0, loop_n, 1):
                emit_body()
        else:
            emit_body()

    nc.compile()
    return nc


_NC_CACHE = {}


def _get_nc(S, D, HPC):
    key = (S, D, HPC)
    if key not in _NC_CACHE:
        _NC_CACHE[key] = build_attention(S, D, HPC)
    return _NC_CACHE[key]


def _prep_batch_x(q_seq, k_seq, v_seq, b, D):
    """Per-batch fp16 x^T shards (shared by the 2 head-group cores)."""
    DC = D // P

    e4m3 = ml_dtypes.float8_e4m3

    def xt(x):  # [S, D] -> [P, 2, DC, S]; 4x split into fp8 hi+lo
        t = np.ascontiguousarray(
            (XS * x).T.reshape(DC, P, -1).transpose(1, 0, 2),
            dtype=np.float32)
        hi = t.astype(e4m3)
        lo = (t - hi.astype(np.float32)).astype(e4m3)
        return np.ascontiguousarray(np.stack([hi, lo], axis=1))

    return {"xq": xt(q_seq[b]), "xk": xt(k_seq[b]), "xv": xt(v_seq[b])}


def _prep_w(WQ, WK, WV, hg, HPC, D):
    """Per-head-group fp16 weight shards."""
    DC = D // P
    CW = HPC * DH

    e4m3 = ml_dtypes.float8_e4m3

    def wslice(w):  # [D, out] -> [P, 2, DC, CW]; 64x split into fp8 hi+lo
        t = np.ascontiguousarray(
            (WSC * w[:, hg * CW : (hg + 1) * CW])
            .reshape(DC, P, CW).transpose(1, 0, 2), dtype=np.float32)
        hi = t.astype(e4m3)
        lo = (t - hi.astype(np.float32)).astype(e4m3)
        return np.ascontiguousarray(np.stack([hi, lo], axis=1))

    return {"wq": wslice(WQ), "wk": wslice(WK), "wv": wslice(WV)}


def _prep_core_inputs(q_seq, k_seq, v_seq, WQ, WK, WV, b, hg, HPC, D):
    """Host-side shard prep for core (batch b, head group hg)."""
    m = _prep_batch_x(q_seq, k_seq, v_seq, b, D)
    m.update(_prep_w(WQ, WK, WV, hg, HPC, D))
    return m


def kernel(q_seq, k_seq, v_seq, WQ, WK, WV, _trace=False):
    q_seq = np.asarray(q_seq, dtype=np.float32)
    k_seq = np.asarray(k_seq, dtype=np.float32)
    v_seq = np.asarray(v_seq, dtype=np.float32)
    WQ = np.asarray(WQ, dtype=np.float32)
    WK = np.asarray(WK, dtype=np.float32)
    WV = np.asarray(WV, dtype=np.float32)

    B, S, D = q_seq.shape
    NB_HEAD = WQ.shape[1] // DH
    n_cores = 8
    groups_per_batch = n_cores // B          # 2 head groups
    HPC = NB_HEAD // groups_per_batch        # 8 heads per core
    CW = HPC * DH

    nc = _get_nc(S, D, HPC)

    xmaps = {b: _prep_batch_x(q_seq, k_seq, v_seq, b, D) for b in range(B)}
    wmaps = {hg: _prep_w(WQ, WK, WV, hg, HPC, D) for hg in range(groups_per_batch)}
    in_maps = []
    for core in range(n_cores):
        b, hg = core // groups_per_batch, core % groups_per_batch
        in_maps.append({**xmaps[b], **wmaps[hg]})

    res = run_bass_kernel_spmd(
        nc, in_maps, core_ids=list(range(n_cores)), trace=_trace,
        **({"trace_cores": [0], } if _trace else {}),
    )
    if _trace:
        print(f"HW exec time: {res.exec_time_ns} ns")
        if res.instructions_and_trace:
            print("trace:", res.instructions_and_trace[1])

    out = np.empty((B, S, NB_HEAD * DH), dtype=np.float32)
    for core in range(n_cores):
        b, hg = core // groups_per_batch, core % groups_per_batch
        # device output: [HPC, P, NQ, 4, DH]; q = qb*512 + qc*128 + p
        ot = res.results[core]["out"]
        ot = ot.transpose(2, 3, 1, 0, 4).reshape(S, CW)
        out[b, :, hg * CW : (hg + 1) * CW] = ot
    return out


# revision 44
# speedup vs baseline: 1.0440x; 1.0037x over previous
"""Multi-head attention Bass kernel for Trainium2, SPMD over 8 NeuronCores.

Problem: B=4, S=2048, D=1024, 16 heads x 64. Sharding: core = (batch b, head-group hg)
with b in 0..3, hg in 0..1 -> each core computes 8 heads of one batch.

Design (cost-model driven, fp16 end-to-end):
  - ScalarE's exp is the hard floor: 256 activations of [128(k), 2(head),
    512(q)] PSUM fp32 -> p fp16, ~1.04us each = ~266us. Everything else is
    arranged to hide under it.
  - scores (PE): per (kc, head) one fp16 matmul K=64 -> s[k, q] in PSUM.
  - AV is Q-MAJOR: O[q, dh] accumulates with M=128 q-positions on PSUM
    partitions and only N=66 columns (64 dh + a ones column that picks up
    the softmax denominator for free + 1 pad for 8B alignment). lhsT is
    the p tile (stationary), rhs is V-augmented [k, 66]. Cost: 66 cycles
    per (kc, head, q-128-chunk) -> ~58us total, half of the k-major form,
    and the denominator lands per-partition-aligned with q so normalize is
    reciprocal + tensor_scalar_mul per chunk - no cross-partition traffic.
  - Four accumulation chains share each PSUM bank; only the chain writing
    first uses start=True (clears the whole bank's has_written bits), the
    others overwrite-where-unset.
  - All deferred work (V projection chunks, AV+finalize, later Q/K
    projections) sits in queues annotated with the earliest "slot" (ACT
    count) at which its input DMA will have landed, so a not-yet-ready
    instruction never enters the PE FIFO ahead of the scores matmuls that
    feed ScalarE. AV closures additionally gate on their V chunk being
    emitted; finalize closures ride the same queue so o_ps frees in order.
  - inputs stream as column-sliced DMAs in consumption order (the DMA
    engine pool is serial in the cost model): wq, wk, xk0, xq0, xk1, xv0,
    xk2, xv1, xk3, xq1, xv2, xv3, xq2, xq3.

PSUM (8 banks): scores 2bufs x [128,2,512] f32 = 4, O accum 2 (2qc x 2h x 66
x 2 banks), projection staging 2.
"""
import numpy as np
import ml_dtypes
from contextlib import ExitStack

import concourse.tile as tile
import concourse.mybir as mybir
from concourse import bacc
from concourse.bass_utils import run_bass_kernel_spmd

P = 128
DH = 64
F16 = mybir.dt.float16
F32 = mybir.dt.float32
FP8 = mybir.dt.float8e4
DR = mybir.MatmulPerfMode.DoubleRow
XS, WSC = 4.0, 64.0

AV_START = 2          # earliest kc for AV draining (first block)
AV_RATE = 6           # max AV/finalize closures drained per kc
V_RATE = 6            # max V-projection closures drained per kc
# earliest global slot for V chunk group c//4 (when xv quarter c//4 landed)
V_SLOTS = (9, 12, 15, 18)
PQ_RATE = 5           # max projection closures drained per kc


def build_attention(S=2048, D=1024, HPC=8, loop_n=1, pops=PQ_RATE):
    """Build the per-core SPMD program. HPC = heads per core (even).

    loop_n > 1 wraps the whole body in a hardware loop (for timing)."""
    DC = D // P        # D chunks of 128
    KC = S // P        # k chunks of 128
    NQ = S // 512      # q blocks of 512
    HP = HPC // 2      # head pairs
    CW = HPC * DH      # core output width
    ACT_SCALE = 1.0 / float(np.sqrt(DH)) / (XS * XS * WSC * WSC)
    DC2 = DC // 2

    nc = bacc.Bacc("TRN2")
    xd, wd = {}, {}
    for n in ("q", "k", "v"):
        xd[n] = nc.dram_tensor("x" + n, [P, 2, DC, S], FP8,
                               kind="ExternalInput")
        wd[n] = nc.dram_tensor("w" + n, [P, 2, DC, CW], FP8,
                               kind="ExternalInput")
    out = nc.dram_tensor("out", [HPC, P, NQ, 4, DH], F32, kind="ExternalOutput")

    with tile.TileContext(nc) as tc, ExitStack() as ctx:
        xpool = ctx.enter_context(tc.tile_pool(name="x", bufs=1))
        wpool = ctx.enter_context(tc.tile_pool(name="w", bufs=1))
        vpool = ctx.enter_context(tc.tile_pool(name="v", bufs=1))
        qkpool = ctx.enter_context(tc.tile_pool(name="qk", bufs=3))
        ppool = ctx.enter_context(tc.tile_pool(name="p", bufs=20))
        rpool = ctx.enter_context(tc.tile_pool(name="r", bufs=4))
        opool = ctx.enter_context(tc.tile_pool(name="ob", bufs=2))
        otpool = ctx.enter_context(tc.tile_pool(name="ot", bufs=2))
        ps_s = ctx.enter_context(tc.tile_pool(name="ps_s", bufs=2, space="PSUM"))
        ps_o = ctx.enter_context(tc.tile_pool(name="ps_o", bufs=1, space="PSUM"))
        ps_m = ctx.enter_context(tc.tile_pool(name="ps_m", bufs=2, space="PSUM"))

        xs, ws = {}, {}
        vta = None
        slot = [0]           # global ACT counter
        vta_done = [False] * KC

        def emit_loads():
            nonlocal vta
            for name in ("q", "k", "v"):
                ws[name] = wpool.tile([P, 2, DC, CW], FP8, tag="w" + name,
                                      name="w" + name)
                xs[name] = xpool.tile([P, 2, DC, S], FP8, tag="x" + name,
                                      name="x" + name)

            def ld(n, c0, c1):
                nc.sync.dma_start(xs[n][:, :, :, c0:c1],
                                  xd[n][:, :, :, c0:c1])

            def ldw(n, c0, c1):
                nc.sync.dma_start(ws[n][:, :, :, c0:c1],
                                  wd[n][:, :, :, c0:c1])

            # DMA order = consumption order (DMA engine pool is serial).
            # hp0's weight columns first; the rest of W after the k/v bulk.
            ldw("q", 0, P)
            ldw("k", 0, P)
            ld("q", 0, 512)
            ld("k", 0, 512)
            ld("k", 512, 1024)
            ld("k", 1024, 1536)
            ldw("v", 0, CW)
            ld("k", 1536, 2048)
            ld("v", 0, 512)
            ld("q", 512, 1024)
            ld("v", 512, 1024)
            ld("v", 1024, 1536)
            ld("v", 1536, 2048)
            ldw("q", P, CW)
            ldw("k", P, CW)
            ld("q", 1024, 1536)
            ld("q", 1536, 2048)
            # V-augmented rhs: [kpos, kc, ch, 66] = V | 1.0 | 0 pad
            vta = vpool.tile([P, KC, HPC, 66], F16, tag="V", name="vta")
            nc.vector.memset(vta[:, :, :, 64], XS * XS * WSC * WSC / 256.0)
            nc.vector.memset(vta[:, :, :, 65], 0.0)

        CROSS = ((0, 0), (0, 1), (1, 0))

        def v_closures(kc, min_slot):
            pstate = {}
            n12 = 3 * DC2

            def mk(i):
                ci, dc2 = divmod(i, DC2)
                a, b = CROSS[ci]

                def f():
                    if i == 0:
                        pstate["pv"] = ps_m.tile([P, 512], F32,
                                                 tag="proj", name="pv")
                    nc.tensor.matmul(
                        pstate["pv"][:, :CW],
                        xs["v"][:, a, 2 * dc2 : 2 * dc2 + 2,
                                kc * P : (kc + 1) * P],
                        ws["v"][:, b, 2 * dc2 : 2 * dc2 + 2, :],
                        start=(i == 0),
                        stop=(i == n12 - 1),
                        perf_mode=DR,
                    )
                    if i == n12 - 1:
                        nc.vector.tensor_copy(
                            vta[:, kc, :, 0:DH],
                            pstate["pv"][:, :CW].rearrange(
                                "p (h d) -> p h d", d=DH),
                        )
                        vta_done[kc] = True
                return f

            return [(min_slot, mk(i)) for i in range(n12)]

        def new_qk(which):
            return qkpool.tile([P, S], F16, tag=which, name=which + "t")

        def proj_qk_chunk(t, which, hp, qb):
            pp = ps_m.tile([P, 512], F32, tag="proj", name="pp")
            n12 = 3 * DC2
            for i in range(n12):
                ci, dc2 = divmod(i, DC2)
                a, b = CROSS[ci]
                nc.tensor.matmul(
                    pp[:],
                    ws[which][:, b, 2 * dc2 : 2 * dc2 + 2,
                              hp * P : (hp + 1) * P],
                    xs[which][:, a, 2 * dc2 : 2 * dc2 + 2,
                              qb * 512 : (qb + 1) * 512],
                    start=(i == 0),
                    stop=(i == n12 - 1),
                    perf_mode=DR,
                )
            nc.vector.tensor_copy(t[:, qb * 512 : (qb + 1) * 512], pp[:])

        def chunk_closures(t, which, hp, qb, min_slot, c0=0, c1=512):
            """(min_slot, closure) items: one per matmul; last also
            evacuates. c0:c1 select columns within the 512-wide chunk."""
            pstate = {}
            w = c1 - c0
            n12 = 3 * DC2

            def mk(i):
                ci, dc2 = divmod(i, DC2)
                a, b = CROSS[ci]

                def f():
                    if i == 0:
                        pstate["pp"] = ps_m.tile([P, 512], F32,
                                                 tag="proj", name="pp")
                    nc.tensor.matmul(
                        pstate["pp"][:, 0:w],
                        ws[which][:, b, 2 * dc2 : 2 * dc2 + 2,
                                  hp * P : (hp + 1) * P],
                        xs[which][:, a, 2 * dc2 : 2 * dc2 + 2,
                                  qb * 512 + c0 : qb * 512 + c1],
                        start=(i == 0),
                        stop=(i == n12 - 1),
                        perf_mode=DR,
                    )
                    if i == n12 - 1:
                        nc.vector.tensor_copy(
                            t[:, qb * 512 + c0 : qb * 512 + c1],
                            pstate["pp"][:, 0:w])
                return f

            return [(min_slot, mk(i)) for i in range(n12)]

        def drain(q, budget, gate=None):
            while budget and q:
                head = q[0]
                if head[0] is not None and head[0] > slot[0]:
                    break
                if gate is not None and not gate(head):
                    break
                q.pop(0)[1]()
                budget -= 1

        def attn_block(hp, qb, qt, kt, proj_q, v_q, av_q, kt_done, q0_q, first_hp=False, last=False):
            # o banks: [128(q), 2(qc half), 2(head), 66]; qc 0,1 -> bank A,
            # qc 2,3 -> bank B
            o_ps = [ps_o.tile([P, 2, 2, 66], F32, tag=f"O{i}", name=f"o{i}")
                    for i in (0, 1)]

            def emit_scores(kc):
                s = ps_s.tile([P, 2, 512], F32, tag="S", name="s")
                for h in (0, 1):
                    nc.tensor.matmul(
                        s[:, h, :],
                        kt[h * DH : (h + 1) * DH, kc * P : (kc + 1) * P],
                        qt[h * DH : (h + 1) * DH, qb * 512 : (qb + 1) * 512],
                        start=True,
                        stop=True,
                    )
                return s

            def mk_av(kc, pt):
                def f():
                    for qc in range(4):
                        for h in (0, 1):
                            nc.tensor.matmul(
                                o_ps[qc // 2][:, qc % 2, h, :],
                                pt[:, h, qc * P : (qc + 1) * P],
                                vta[:, kc, hp * 2 + h, :],
                                start=(kc == 0 and qc % 2 == 0 and h == 0),
                                stop=(kc == KC - 1),
                                skip_group_check=(qc + h > 0),
                            )
                return f

            def finalize():
                ot = otpool.tile([P, 4, 2, DH], F32, tag="ot", name="ot")
                for i in (0, 1):
                    osb = opool.tile([P, 2, 2, 66], F32, tag="osb", name="osb")
                    nc.vector.tensor_copy(osb[:], o_ps[i][:])
                    for j in (0, 1):
                        for h in (0, 1):
                            rt = rpool.tile([P, 1], F32, tag="rt", name="rt")
                            nc.vector.reciprocal(rt[:], osb[:, j, h, 64:65])
                            nc.vector.tensor_scalar_mul(
                                ot[:, 2 * i + j, h, :], osb[:, j, h, 0:DH],
                                rt[:, 0:1])
                for h in (0, 1):
                    ch = hp * 2 + h
                    nc.sync.dma_start(out[ch, :, qb, :, :], ot[:, :, h, :])

            def gate_av(head):
                kc = head[2]
                return kc is None or vta_done[kc]

            s_cur = emit_scores(0)
            for kc in range(KC):
                pt = ppool.tile([P, 2, 512], F16, tag="p", name="pt")
                nc.scalar.activation(
                    pt[:], s_cur[:],
                    mybir.ActivationFunctionType.Exp,
                    scale=ACT_SCALE)
                slot[0] += 1
                if kc + 1 < KC:
                    if first_hp and qb == 0:
                        # force-drain deferred kt work until the columns the
                        # next scores matmul reads have been projected
                        while kt_done[0] < (kc + 2) * P and proj_q:
                            assert proj_q[0][2] == -1
                            proj_q.pop(0)[1]()
                    s_cur = emit_scores(kc + 1)
                drain(v_q, V_RATE)
                av_q.append((None, mk_av(kc, pt), kc))
                drain(av_q, len(av_q) if last else AV_RATE, gate=gate_av)
                drain(q0_q, 2)
                drain(proj_q, pops)
            av_q.append((None, finalize, None))

        def emit_body():
            emit_loads()
            qt = new_qk("q")
            kt = new_qk("k")
            # warm the PE p-state during the input-DMA wait: dummy
            # matmuls over the zeroed vta keep the ramp model at full speed
            # for the first real projections
            oc = vta[:, 0, :, 64:66]   # [P, HPC, 2] initialized slice
            for i in range(50):
                wp = ps_m.tile([P, 512], F32, tag="proj", name="wp")
                nc.tensor.matmul(
                    wp[0:1, 0:256],
                    oc[:, 0, 0:1],
                    oc.to_broadcast((P, HPC, 2, 16)),
                    start=True, stop=True)
            # prologue: just enough projection for the first scores:
            # Q chunk 0 (xq0 lands first), then K chunk 0 cols 0:256.
            proj_qk_chunk(qt, "q", 0, 0)
            for _s, f in chunk_closures(kt, "k", 0, 0, 0, 0, 256):
                f()
            # deferred, force-drained ahead of the scores that read them
            # (tag -1 entries carry kt columns; kt_cols tracks progress)
            proj_q = []
            kt_done = [256]

            def mark(cols):
                def g():
                    kt_done[0] = cols
                return g

            proj_q += [(0, f, -1) for _s, f in
                       chunk_closures(kt, "k", 0, 0, 0, 256, 512)]
            proj_q.append((0, mark(512), -1))
            proj_q += [(0, f, -1) for _s, f in
                       chunk_closures(kt, "k", 0, 1, 0)]
            proj_q.append((0, mark(1024), -1))
            proj_q += [(4, f, -1) for _s, f in
                       chunk_closures(kt, "k", 0, 2, 4)]
            proj_q.append((4, mark(1536), -1))
            proj_q += [(8, f, -1) for _s, f in
                       chunk_closures(kt, "k", 0, 3, 8)]
            proj_q.append((8, mark(2048), -1))
            q0_q = []
            for qb, ms in ((1, 15), (2, 29), (3, 32)):
                q0_q += [(s0, f, qb) for s0, f in
                         chunk_closures(qt, "q", 0, qb, ms)]
            v_q = []
            for kc in range(KC):
                v_q += v_closures(kc, V_SLOTS[kc // 4] + 2 * (kc % 4))
            av_q = []

            # prefetch queues for hp 1..3, tagged with their hp so the
            # boundary flush can force-complete exactly what's needed
            qts = {0: (qt, kt)}
            for hpn in range(1, HP):
                base = (27, 56, 104, 170)[hpn]
                qts[hpn] = (new_qk("q"), new_qk("k"))
                for qb in range(NQ):
                    proj_q += [(max(s0, base), f, hpn) for s0, f in
                               chunk_closures(qts[hpn][1], "k", hpn, qb, 0)]
                for qb in range(NQ):
                    proj_q += [(max(s0, base, 26), f, hpn) for s0, f in
                               chunk_closures(qts[hpn][0], "q", hpn, qb, 0)]
            for hp in range(HP):
                qt, kt = qts[hp]
                for qb in range(NQ):
                    if hp == 0:
                        # hp0's qt chunk qb must be fully projected before
                        # this block's scores read it
                        while q0_q and q0_q[0][2] <= qb:
                            q0_q.pop(0)[1]()
                    attn_block(hp, qb, qt, kt, proj_q, v_q, av_q, kt_done,
                               q0_q if hp == 0 else [],
                               first_hp=(hp == 0),
                               last=(hp == HP - 1 and qb == NQ - 1))
                # next head pair's projections must be fully emitted before
                # its attention reads them
                if hp + 1 < HP:
                    while proj_q and proj_q[0][2] <= hp + 1:
                        proj_q.pop(0)[1]()
            while v_q:
                v_q.pop(0)[1]()
            while av_q:
                av_q.pop(0)[1]()

        if loop_n > 1:
            with tc.For_i(0, loop_n, 1):
                emit_body()
        else:
            emit_body()

    nc.compile()
    return nc


_NC_CACHE = {}


def _get_nc(S, D, HPC):
    key = (S, D, HPC)
    if key not in _NC_CACHE:
        _NC_CACHE[key] = build_attention(S, D, HPC)
    return _NC_CACHE[key]


def _prep_batch_x(q_seq, k_seq, v_seq, b, D):
    """Per-batch fp16 x^T shards (shared by the 2 head-group cores)."""
    DC = D // P

    e4m3 = ml_dtypes.float8_e4m3

    def xt(x):  # [S, D] -> [P, 2, DC, S]; 4x split into fp8 hi+lo
        t = np.ascontiguousarray(
            (XS * x).T.reshape(DC, P, -1).transpose(1, 0, 2),
            dtype=np.float32)
        hi = t.astype(e4m3)
        lo = (t - hi.astype(np.float32)).astype(e4m3)
        return np.ascontiguousarray(np.stack([hi, lo], axis=1))

    return {"xq": xt(q_seq[b]), "xk": xt(k_seq[b]), "xv": xt(v_seq[b])}


def _prep_w(WQ, WK, WV, hg, HPC, D):
    """Per-head-group fp16 weight shards."""
    DC = D // P
    CW = HPC * DH

    e4m3 = ml_dtypes.float8_e4m3

    def wslice(w):  # [D, out] -> [P, 2, DC, CW]; 64x split into fp8 hi+lo
        t = np.ascontiguousarray(
            (WSC * w[:, hg * CW : (hg + 1) * CW])
            .reshape(DC, P, CW).transpose(1, 0, 2), dtype=np.float32)
        hi = t.astype(e4m3)
        lo = (t - hi.astype(np.float32)).astype(e4m3)
        return np.ascontiguousarray(np.stack([hi, lo], axis=1))

    return {"wq": wslice(WQ), "wk": wslice(WK), "wv": wslice(WV)}


def _prep_core_inputs(q_seq, k_seq, v_seq, WQ, WK, WV, b, hg, HPC, D):
    """Host-side shard prep for core (batch b, head group hg)."""
    m = _prep_batch_x(q_seq, k_seq, v_seq, b, D)
    m.update(_prep_w(WQ, WK, WV, hg, HPC, D))
    return m


def kernel(q_seq, k_seq, v_seq, WQ, WK, WV, _trace=False):
    q_seq = np.asarray(q_seq, dtype=np.float32)
    k_seq = np.asarray(k_seq, dtype=np.float32)
    v_seq = np.asarray(v_seq, dtype=np.float32)
    WQ = np.asarray(WQ, dtype=np.float32)
    WK = np.asarray(WK, dtype=np.float32)
    WV = np.asarray(WV, dtype=np.float32)

    B, S, D = q_seq.shape
    NB_HEAD = WQ.shape[1] // DH
    n_cores = 8
    groups_per_batch = n_cores // B          # 2 head groups
    HPC = NB_HEAD // groups_per_batch        # 8 heads per core
    CW = HPC * DH

    nc = _get_nc(S, D, HPC)

    xmaps = {b: _prep_batch_x(q_seq, k_seq, v_seq, b, D) for b in range(B)}
    wmaps = {hg: _prep_w(WQ, WK, WV, hg, HPC, D) for hg in range(groups_per_batch)}
    in_maps = []
    for core in range(n_cores):
        b, hg = core // groups_per_batch, core % groups_per_batch
        in_maps.append({**xmaps[b], **wmaps[hg]})

    res = run_bass_kernel_spmd(
        nc, in_maps, core_ids=list(range(n_cores)), trace=_trace,
        **({"trace_cores": [0], } if _trace else {}),
    )
    if _trace:
        print(f"HW exec time: {res.exec_time_ns} ns")
        if res.instructions_and_trace:
            print("trace:", res.instructions_and_trace[1])

    out = np.empty((B, S, NB_HEAD * DH), dtype=np.float32)
    for core in range(n_cores):
        b, hg = core // groups_per_batch, core % groups_per_batch
        # device output: [HPC, P, NQ, 4, DH]; q = qb*512 + qc*128 + p
        ot = res.results[core]["out"]
        ot = ot.transpose(2, 3, 1, 0, 4).reshape(S, CW)
        out[b, :, hg * CW : (hg + 1) * CW] = ot
    return out
